# revision 1
# baseline (speedup 1.0000x reference)
"""Trainium2 Bass kernel for nn_AbstractFullyConnected (DeepPoly abstract
interpretation of a 5-layer MLP, FC = [784, 2048, 2048, 2048, 2048, 10]).

Strategy (8 NeuronCores, tensor-parallel):
  * Each layer-i bound computation is a back-substitution chain of GEMMs.
    The chain state is kept TRANSPOSED ("G-form": G[k, r] = M[r, k]) so the
    natural [out, in] weight layout serves directly as the matmul stationary
    operand (out = lhsT.T @ rhs) and no per-step transposes are needed.
  * The chain's output rows (layer-i out dim, 2048) are sharded 256/core; the
    low and high chains are stacked along the free dim (2 x 256 = 512 cols),
    so every chain GEMM is [K=2048] x [M=2048 or 784] x [N=512] per core.
  * ReLU back-substitution (sign-dependent column scaling) is fused into the
    PSUM->SBUF drain on the Scalar engine: both DeepPoly coefficient vectors
    are non-negative, so  Relu(c (.) G) = c*max(G,0)  and  Relu(-c (.) G) =
    -c*min(G,0)  give the pre-scaled positive/negative parts in one pass.
  * After each layer only the bound VECTORS (x/low/high, 256/core) are
    AllGathered (~3 KB) - no large collectives anywhere.
  * Big chain GEMMs run as float32r (full PE rate, ~fp22); all small matmuls
    (bias columns, boxes, matvecs, the whole 10-wide layer-5 chain) run as
    true fp32. Accumulation is fp32 in PSUM.
  * Layer-5 chain (10 outputs) shards each GEMM's out dim across cores with a
    small AllGather per step.
"""

import contextlib
import os

import numpy as np

MEAN, STD = 0.1307, 0.3081
N_CORES = 8
R = 256          # chain rows per core (2048 / 8)
NT = 16          # k-tiles for 2048
NT1 = 7          # k-tiles for 784 (padded to 896)
IN1P = 896

_CACHE = {}


# ----------------------------------------------------------------------------
# walrus in this container supports only ONE sync-wait per instruction; hoist
# extra waits emitted by the Tile scheduler into standalone single-wait
# EventSemaphore instructions placed just before the owning instruction.
# ----------------------------------------------------------------------------
def _split_multiwaits(nc):
    import concourse.mybir as mybir

    n = 0
    for f in nc.m.functions:
        for b in f.blocks:
            insts = list(b.instructions)
            if not any(
                (i.sync_info is not None and len(i.sync_info.on_wait) > 1)
                for i in insts
            ):
                continue
            new = []
            for i in insts:
                si = i.sync_info
                if si is not None and len(si.on_wait) > 1:
                    waits = list(si.on_wait)
                    for k, w in enumerate(waits[:-1]):
                        ev = mybir.InstEventSemaphore(
                            name=f"{i.name}_hw{k}", ins=[], outs=[]
                        )
                        ev.engine = i.engine
                        ev.sync_info = mybir.SyncInfo(on_wait=[w], on_update=[])
                        new.append(ev)
                        n += 1
                    i.sync_info = mybir.SyncInfo(
                        on_wait=[waits[-1]], on_update=list(si.on_update)
                    )
                new.append(i)
            b.instructions = new
    return n


def build_nc():
    KREP = int(os.environ.get("BASS_KREP", "1"))
    NOCC = bool(int(os.environ.get("BASS_NOCC", "0")))
    import concourse.bass as bass
    import concourse.mybir as mybir
    import concourse.tile as tile

    F32 = mybir.dt.float32
    F32R = mybir.dt.float32r
    AF = mybir.ActivationFunctionType
    ALU = mybir.AluOpType

    nc = bass.Bass("TRN2", target_bir_lowering=False, debug=False,
                   num_devices=N_CORES)

    # ---------------- DRAM I/O ----------------
    def din(name, shape):
        return nc.dram_tensor(name, shape, F32, kind="ExternalInput").ap()

    W_t = {l: din(f"W{l}t", [NT1 if l == 1 else NT, 2048, 128]) for l in (1, 2, 3)}
    Wsh = {l: din(f"W{l}sh", [2, 2048, 128]) for l in (2, 3, 4)}
    W1sh5 = din("W1sh5", [1, 2048, 128])
    GT = {i: din(f"G{i}T", [2048, R]) for i in (2, 3, 4)}
    G1T = din("G1T", [IN1P, R])
    W5T = din("W5T", [2048, 10])
    bc = {l: din(f"bc{l}", [16, 128]) for l in (1, 2, 3, 4)}
    bsh = {i: din(f"b{i}sh", [128, 2]) for i in (1, 2, 3, 4)}
    b5d = din("b5", [10, 1])
    xlh = din("xlh", [3, NT1, 128])  # raw padded x / low / high

    out_d = nc.dram_tensor("out", [3, 10], F32, kind="ExternalOutput").ap()
    dbg_d = nc.dram_tensor("dbg", [16, 128, 16], F32, kind="ExternalOutput").ap()
    _dbg_n = [0]
    def dbg_dump(tile_ap):
        s = _dbg_n[0]; _dbg_n[0] += 1
        if s >= 16: return
        w = tile_ap.shape[-1]
        nc.sync.dma_start(dbg_d[s, :, 0:w], tile_ap)


    # internal DRAM for collectives
    ag_in = {i: nc.dram_tensor(f"ag{i}_in", [2, 384], F32).ap()
             for i in (1, 2, 3, 4)}
    ag_out = {i: nc.dram_tensor(f"ag{i}_out", [16, 384], F32,
                                addr_space="Shared").ap()
              for i in (1, 2, 3, 4)}
    ag5_in = {l: nc.dram_tensor(f"ag5_{l}_in", [2, 2560], F32).ap()
              for l in (4, 3, 2)}
    ag5_out = {l: nc.dram_tensor(f"ag5_{l}_out", [16, 2560], F32,
                                 addr_space="Shared").ap()
               for l in (4, 3, 2)}
    ag5f_in = nc.dram_tensor("ag5f_in", [1, 2560], F32).ap()
    ag5f_out = nc.dram_tensor("ag5f_out", [8, 2560], F32,
                              addr_space="Shared").ap()
    rg = [list(range(N_CORES))]

    with tile.TileContext(nc) as tc, contextlib.ExitStack() as est:
        pool_c = est.enter_context(tc.tile_pool(name="const", bufs=1))
        pool_git = est.enter_context(tc.tile_pool(name="git", bufs=2))
        pool_w = est.enter_context(tc.tile_pool(name="wstream", bufs=3))
        pool_gs = est.enter_context(tc.tile_pool(name="gs", bufs=2))
        pool_abcd = est.enter_context(tc.tile_pool(name="abcd", bufs=4))
        pool_misc = est.enter_context(tc.tile_pool(name="misc", bufs=2))
        pool_ps = est.enter_context(tc.tile_pool(name="ps", bufs=4, space="PSUM"))
        pool_bb = est.enter_context(tc.tile_pool(name="bb", bufs=2, space="PSUM"))
        pool_ps5 = est.enter_context(tc.tile_pool(name="ps5", bufs=1, space="PSUM"))

        # ---------------- constants / vectors ----------------
        def vtile(name, w=16):
            return pool_c.tile([128, w], F32, tag=name, name=name)

        x0n, lo0n, hi0n = vtile("x0n", NT1), vtile("lo0n", NT1), vtile("hi0n", NT1)
        nlo0n, nhi0n = vtile("nlo0n", NT1), vtile("nhi0n", NT1)
        raw = pool_c.tile([128, 3 * NT1], F32, tag="rawxlh", name="rawxlh")
        nc.sync.dma_start(
            raw[:, :].rearrange("p (k t) -> p k t", k=3),
            xlh.rearrange("k t p -> p k t"),
        )
        for j, dst in enumerate((x0n, lo0n, hi0n)):
            nc.scalar.activation(dst[:, :], raw[:, j * NT1:(j + 1) * NT1],
                                 AF.Copy, bias=-MEAN / STD, scale=1.0 / STD)
        nc.vector.tensor_scalar_mul(nlo0n[:, :], lo0n[:, :], -1.0)
        nc.vector.tensor_scalar_mul(nhi0n[:, :], hi0n[:, :], -1.0)

        bcs = {}
        for l in (1, 2, 3, 4):
            t = pool_c.tile([128, 16], F32, tag=f"bc{l}", name=f"bc{l}")
            nc.sync.dma_start(t[:, :], bc[l].rearrange("t p -> p t"))
            bcs[l] = t
        bshs = {}
        for i in (1, 2, 3, 4):
            t = pool_c.tile([128, 2], F32, tag=f"bsh{i}", name=f"bsh{i}")
            nc.sync.dma_start(t[:, :], bsh[i][:, :])
            bshs[i] = t
        b5t = pool_c.tile([10, 1], F32, tag="b5t", name="b5t")
        nc.sync.dma_start(b5t[:, :], b5d[:, :])

        w5buf = pool_c.tile([128, 16 * 10], F32, tag="w5buf", name="w5buf")
        nc.sync.dma_start(
            w5buf[:, :].rearrange("p (t c) -> p t c", t=16),
            W5T.rearrange("(t p) c -> p t c", p=128),
        )
        g1t = pool_c.tile([128, NT1 * R], F32, tag="g1t", name="g1t")
        nc.sync.dma_start(
            g1t[:, :].rearrange("p (t c) -> p t c", t=NT1),
            G1T.rearrange("(t p) c -> p t c", p=128),
        )

        # per-layer relu coefficient tiles (filled after each layer)
        coef = {}
        for i in (1, 2, 3, 4):
            coef[i] = {k: pool_c.tile([128, 16], F32, tag=f"cf{i}{k}", name=f"cf{i}{k}")
                       for k in ("c1", "c2", "nc1", "nc2", "rhv", "nrhv", "xr")}

        # ---------------- helpers ----------------
        class BiasCols:
            """bias accumulation columns in one PSUM bank.
            cols 0,1: low m0/m1 | 2,3: high | 4,5: x"""

            def __init__(self, np_part=128):
                self.t = pool_bb.tile([128, 8], F32, tag="bb", name="bb")
                # start=True zeroes the WHOLE PSUM bank on this HW, so emit
                # exactly one start for the bank; later first-touches rely on
                # the bank-wide has_written clear (first write = overwrite).
                self.bank_first = True
                self.np_part = np_part

            def mm(self, col, lhsT, rhs, stop=False):
                nc.tensor.matmul(
                    self.t[0:self.np_part, col:col + 1], lhsT, rhs,
                    start=self.bank_first, stop=stop,
                )
                self.bank_first = False

        def relu_pass(dst, src, scale):
            nc.scalar.activation(dst, src, AF.Relu, scale=scale)

        def do_allgather(in_ap, out_ap, rows_per_rank):
            if NOCC:
                # timing-only stub: copy own shard into its slot
                nc.sync.dma_start(out_ap[0:rows_per_rank], in_ap[0:rows_per_rank])
            else:
                nc.gpsimd.collective_compute(
                    "AllGather", ALU.bypass, replica_groups=rg,
                    ins=[in_ap], outs=[out_ap])

        def gather_layer(i, bbias):
            """add b_i shard, DMA out, AllGather, read back, compute coeffs."""
            sh = pool_misc.tile([128, 6], F32, tag="sh", name="sh")
            for m in range(2):
                for c in (0, 2, 4):
                    nc.vector.tensor_tensor(
                        sh[:, c + m:c + m + 1],
                        bbias.t[:, c + m:c + m + 1],
                        bshs[i][:, m:m + 1], ALU.add)
            # shard row jj = [x | lo | hi] each 128 wide
            for m in range(2):
                for kind, c in ((0, 4), (1, 0), (2, 2)):  # x, lo, hi
                    nc.sync.dma_start(
                        ag_in[i][m:m + 1, kind * 128:(kind + 1) * 128]
                        .rearrange("a p -> p a"),
                        sh[:, c + m:c + m + 1])
            do_allgather(ag_in[i][:, :], ag_out[i][:, :], 2)
            xf = pool_misc.tile([128, 16], F32, tag="xf", name="xf")
            lof = pool_misc.tile([128, 16], F32, tag="lof", name="lof")
            hif = pool_misc.tile([128, 16], F32, tag="hif", name="hif")
            for kind, dst in ((0, xf), (1, lof), (2, hif)):
                nc.sync.dma_start(
                    dst[:, :],
                    ag_out[i][:, kind * 128:(kind + 1) * 128]
                    .rearrange("t p -> p t"))
            dbg_dump(xf[:, :]); dbg_dump(lof[:, :]); dbg_dump(hif[:, :])
            compute_coeffs(i, xf, lof, hif)
            dbg_dump(coef[i]["c1"][:, :]); dbg_dump(coef[i]["c2"][:, :])

        _cw = [0]

        def compute_coeffs(i, x, lo, hi):
            C = coef[i]

            def tmp():
                _cw[0] += 1
                return pool_misc.tile([128, 16], F32, tag=f"cw{_cw[0] % 20}", name=f"cw{_cw[0] % 20}")

            tln, thp, m = tmp(), tmp(), tmp()
            nc.vector.tensor_scalar(tln[:, :], lo[:, :], 0.0, None, ALU.is_lt)
            nc.vector.tensor_scalar(thp[:, :], hi[:, :], 0.0, None, ALU.is_gt)
            nc.vector.tensor_tensor(m[:, :], tln[:, :], thp[:, :], ALU.mult)
            d, onem, dsafe = tmp(), tmp(), tmp()
            nc.vector.tensor_tensor(d[:, :], hi[:, :], lo[:, :], ALU.subtract)
            nc.vector.tensor_scalar(onem[:, :], m[:, :], -1.0, 1.0,
                                    ALU.mult, ALU.add)
            nc.vector.tensor_tensor(dsafe[:, :], d[:, :], m[:, :], ALU.mult)
            nc.vector.tensor_tensor(dsafe[:, :], dsafe[:, :], onem[:, :], ALU.add)
            r = tmp()
            nc.vector.reciprocal(r[:, :], dsafe[:, :])
            hr, usm, lh, bhv = tmp(), tmp(), tmp(), tmp()
            nc.vector.tensor_tensor(hr[:, :], hi[:, :], r[:, :], ALU.mult)
            nc.vector.tensor_tensor(usm[:, :], hr[:, :], m[:, :], ALU.mult)
            nc.vector.tensor_tensor(lh[:, :], lo[:, :], hr[:, :], ALU.mult)
            nc.vector.scalar_tensor_tensor(bhv[:, :], lh[:, :], -1.0, m[:, :],
                                           ALU.mult, ALU.mult)
            lo2, hi2, lam = tmp(), tmp(), tmp()
            nc.vector.tensor_tensor(lo2[:, :], lo[:, :], lo[:, :], ALU.mult)
            nc.vector.tensor_tensor(hi2[:, :], hi[:, :], hi[:, :], ALU.mult)
            nc.vector.tensor_tensor(lam[:, :], lo2[:, :], hi2[:, :], ALU.is_le)
            k1 = tmp()
            nc.vector.tensor_tensor(k1[:, :], thp[:, :], onem[:, :], ALU.mult)
            nc.vector.tensor_tensor(C["c1"][:, :], usm[:, :], k1[:, :], ALU.add)
            lamm = tmp()
            nc.vector.tensor_tensor(lamm[:, :], lam[:, :], m[:, :], ALU.mult)
            nc.vector.tensor_tensor(C["c2"][:, :], lamm[:, :], k1[:, :], ALU.add)
            nc.vector.tensor_scalar_mul(C["nc1"][:, :], C["c1"][:, :], -1.0)
            nc.vector.tensor_scalar_mul(C["nc2"][:, :], C["c2"][:, :], -1.0)
            c1z, c1g, rc = tmp(), tmp(), tmp()
            nc.vector.tensor_scalar(c1z[:, :], C["c1"][:, :], 0.0, None,
                                    ALU.is_equal)
            nc.vector.tensor_tensor(c1g[:, :], C["c1"][:, :], c1z[:, :], ALU.add)
            nc.vector.reciprocal(rc[:, :], c1g[:, :])
            nc.vector.tensor_tensor(C["rhv"][:, :], bhv[:, :], rc[:, :], ALU.mult)
            nc.vector.tensor_scalar_mul(C["nrhv"][:, :], C["rhv"][:, :], -1.0)
            nc.scalar.activation(C["xr"][:, :], x[:, :], AF.Relu)

        for _rep in range(KREP):
            # ================= chain 1 (layer 1) =================
            bb1 = BiasCols()
            for t in range(NT1):
                gsl = g1t[:, t * R:(t + 1) * R]
                P1 = pool_abcd.tile([128, R], F32, tag="A", name="A")
                N1 = pool_abcd.tile([128, R], F32, tag="B", name="B")
                relu_pass(P1[:, :], gsl, 1.0)
                nc.vector.tensor_scalar(N1[:, :], gsl, 0.0, -1.0, ALU.min, ALU.mult)
                last = t == NT1 - 1
                for m in range(2):
                    sl = slice(m * 128, (m + 1) * 128)
                    bb1.mm(0 + m, P1[:, sl], lo0n[:, t:t + 1])
                    bb1.mm(0 + m, N1[:, sl], nhi0n[:, t:t + 1], stop=last)
                    bb1.mm(2 + m, P1[:, sl], hi0n[:, t:t + 1])
                    bb1.mm(2 + m, N1[:, sl], nlo0n[:, t:t + 1], stop=last)
                    bb1.mm(4 + m, g1t[:, t * R + m * 128:t * R + (m + 1) * 128],
                           x0n[:, t:t + 1], stop=last)
            gather_layer(1, bb1)

            # ================= chains 2..4 =================
            for i in (2, 3, 4):
                g = pool_git.tile([128, NT * R], F32, tag="git", name="git")
                nc.sync.dma_start(
                    g[:, :].rearrange("p (t c) -> p t c", t=NT),
                    GT[i].rearrange("(t p) c -> p t c", p=128),
                )

                bbx = BiasCols()
                cf = coef[i - 1]
                Gs = pool_gs.tile([128, NT * 512], F32, tag="gs", name="gs")
                for t in range(NT):
                    gsl = g[:, t * R:(t + 1) * R]
                    A = pool_abcd.tile([128, R], F32, tag="A", name="A")
                    B = pool_abcd.tile([128, R], F32, tag="B", name="B")
                    Cc = pool_abcd.tile([128, R], F32, tag="C", name="C")
                    D = pool_abcd.tile([128, R], F32, tag="D", name="D")
                    relu_pass(A[:, :], gsl, cf["c1"][:, t:t + 1])
                    nc.vector.tensor_scalar(B[:, :], gsl, 0.0, cf["nc2"][:, t:t + 1], ALU.min, ALU.mult)
                    relu_pass(Cc[:, :], gsl, cf["c2"][:, t:t + 1])
                    nc.vector.tensor_scalar(D[:, :], gsl, 0.0, cf["nc1"][:, t:t + 1], ALU.min, ALU.mult)
                    nc.vector.tensor_tensor(
                        Gs[:, t * 512:t * 512 + 256].bitcast(F32R),
                        Cc[:, :], D[:, :], ALU.subtract)
                    nc.vector.tensor_tensor(
                        Gs[:, t * 512 + 256:t * 512 + 512].bitcast(F32R),
                        A[:, :], B[:, :], ALU.subtract)
                    for m in range(2):
                        sl = slice(m * 128, (m + 1) * 128)
                        bbx.mm(2 + m, A[:, sl], cf["rhv"][:, t:t + 1])
                        bbx.mm(0 + m, D[:, sl], cf["nrhv"][:, t:t + 1])
                        bbx.mm(4 + m, g[:, t * R + m * 128:t * R + (m + 1) * 128],
                               cf["xr"][:, t:t + 1], stop=(t == NT - 1))

                for l in range(i - 1, 0, -1):
                    # linear bias vs b_l on the current (scaled) state
                    for t in range(NT):
                        for m in range(2):
                            lo_l = Gs[:, t * 512 + m * 128:t * 512 + (m + 1) * 128]
                            hi_l = Gs[:, t * 512 + 256 + m * 128:
                                      t * 512 + 256 + (m + 1) * 128]
                            bbx.mm(0 + m, lo_l, bcs[l][:, t:t + 1])
                            bbx.mm(2 + m, hi_l, bcs[l][:, t:t + 1])
                    nj = NT if l > 1 else NT1
                    Gs_next = (pool_gs.tile([128, NT * 512], F32, tag="gs", name="gs")
                               if l > 1 else None)
                    cfl = coef[l - 1] if l > 1 else None
                    for j in range(nj):
                        wb = pool_w.tile([128, 2048], F32, tag="wb", name="wb")
                        nc.sync.dma_start(
                            wb[:, :].rearrange("p (t c) -> p t c", t=NT).bitcast(F32R),
                            W_t[l][j].rearrange("(t p) c -> p t c", p=128).bitcast(F32R),
                        )
                        ps = pool_ps.tile([128, 512], F32, tag="ps", name="ps")
                        for k in range(NT):
                            nc.tensor.matmul(
                                ps[:, :],
                                wb[:, k * 128:(k + 1) * 128].bitcast(F32R),
                                Gs[:, k * 512:(k + 1) * 512].bitcast(F32R),
                                start=(k == 0), stop=(k == NT - 1))
                        if l > 1:
                            A = pool_abcd.tile([128, R], F32, tag="A", name="A")
                            B = pool_abcd.tile([128, R], F32, tag="B", name="B")
                            Cc = pool_abcd.tile([128, R], F32, tag="C", name="C")
                            D = pool_abcd.tile([128, R], F32, tag="D", name="D")
                            hi_sl, lo_sl = ps[:, 256:512], ps[:, 0:256]
                            relu_pass(A[:, :], hi_sl, cfl["c1"][:, j:j + 1])
                            nc.vector.tensor_scalar(B[:, :], hi_sl, 0.0, cfl["nc2"][:, j:j + 1], ALU.min, ALU.mult)
                            relu_pass(Cc[:, :], lo_sl, cfl["c2"][:, j:j + 1])
                            nc.vector.tensor_scalar(D[:, :], lo_sl, 0.0, cfl["nc1"][:, j:j + 1], ALU.min, ALU.mult)
                            nc.vector.tensor_tensor(
                                Gs_next[:, j * 512:j * 512 + 256].bitcast(F32R),
                                Cc[:, :], D[:, :], ALU.subtract)
                            nc.vector.tensor_tensor(
                                Gs_next[:, j * 512 + 256:j * 512 + 512].bitcast(F32R),
                                A[:, :], B[:, :], ALU.subtract)
                            for m in range(2):
                                sl = slice(m * 128, (m + 1) * 128)
                                bbx.mm(2 + m, A[:, sl], cfl["rhv"][:, j:j + 1])
                                bbx.mm(0 + m, D[:, sl], cfl["nrhv"][:, j:j + 1])
                        else:
                            Ph = pool_abcd.tile([128, R], F32, tag="A", name="A")
                            Nh = pool_abcd.tile([128, R], F32, tag="B", name="B")
                            Pl = pool_abcd.tile([128, R], F32, tag="C", name="C")
                            Nl = pool_abcd.tile([128, R], F32, tag="D", name="D")
                            relu_pass(Ph[:, :], ps[:, 256:512], 1.0)
                            nc.vector.tensor_scalar(Nh[:, :], ps[:, 256:512],
                                                    0.0, -1.0, ALU.min, ALU.mult)
                            relu_pass(Pl[:, :], ps[:, 0:256], 1.0)
                            nc.vector.tensor_scalar(Nl[:, :], ps[:, 0:256],
                                                    0.0, -1.0, ALU.min, ALU.mult)
                            last = j == nj - 1
                            for m in range(2):
                                sl = slice(m * 128, (m + 1) * 128)
                                bbx.mm(0 + m, Pl[:, sl], lo0n[:, j:j + 1])
                                bbx.mm(0 + m, Nl[:, sl], nhi0n[:, j:j + 1], stop=last)
                                bbx.mm(2 + m, Ph[:, sl], hi0n[:, j:j + 1])
                                bbx.mm(2 + m, Nh[:, sl], nlo0n[:, j:j + 1], stop=last)
                    if l > 1:
                        Gs = Gs_next
                gather_layer(i, bbx)

            # ================= chain 5 (all small matmuls: true fp32) ==========
            cf4 = coef[4]
            G5s = pool_c.tile([128, 16 * 20], F32, tag="g5s", name="g5s")
            bb5 = BiasCols(np_part=10)
            for t in range(NT):
                gsl = w5buf[:, t * 10:(t + 1) * 10]
                A = pool_abcd.tile([128, 10], F32, tag="A5", name="A5")
                B = pool_abcd.tile([128, 10], F32, tag="B5", name="B5")
                Cc = pool_abcd.tile([128, 10], F32, tag="C5", name="C5")
                D = pool_abcd.tile([128, 10], F32, tag="D5", name="D5")
                relu_pass(A[:, :], gsl, cf4["c1"][:, t:t + 1])
                nc.vector.tensor_scalar(B[:, :], gsl, 0.0, cf4["nc2"][:, t:t + 1], ALU.min, ALU.mult)
                relu_pass(Cc[:, :], gsl, cf4["c2"][:, t:t + 1])
                nc.vector.tensor_scalar(D[:, :], gsl, 0.0, cf4["nc1"][:, t:t + 1], ALU.min, ALU.mult)
                nc.vector.tensor_tensor(G5s[:, t * 20:t * 20 + 10],
                                        Cc[:, :], D[:, :], ALU.subtract)
                nc.vector.tensor_tensor(G5s[:, t * 20 + 10:t * 20 + 20],
                                        A[:, :], B[:, :], ALU.subtract)
                bb5.mm(2, A[:, :], cf4["rhv"][:, t:t + 1])
                bb5.mm(0, D[:, :], cf4["nrhv"][:, t:t + 1])
                bb5.mm(4, w5buf[:, t * 10:(t + 1) * 10], cf4["xr"][:, t:t + 1],
                       stop=(t == NT - 1))

            for l in (4, 3, 2, 1):
                for t in range(NT):
                    bb5.mm(0, G5s[:, t * 20:t * 20 + 10], bcs[l][:, t:t + 1])
                    bb5.mm(2, G5s[:, t * 20 + 10:t * 20 + 20], bcs[l][:, t:t + 1])
                if l > 1:
                    ps5 = pool_ps5.tile([128, 40], F32, tag="ps5", name="ps5")
                    wb = pool_w.tile([128, 2 * 2048], F32, tag="wb5", name="wb5", bufs=1)
                    nc.sync.dma_start(
                        wb[:, :].rearrange("p (j t c) -> p j t c", j=2, t=NT),
                        Wsh[l].rearrange("j (t p) c -> p j t c", p=128),
                    )
                    for jj in range(2):
                        for k in range(NT):
                            nc.tensor.matmul(
                                ps5[:, jj * 20:(jj + 1) * 20],
                                wb[:, jj * 2048 + k * 128:jj * 2048 + (k + 1) * 128],
                                G5s[:, k * 20:(k + 1) * 20],
                                start=(jj == 0 and k == 0), stop=(k == NT - 1))
                    shc = pool_misc.tile([128, 40], F32, tag="shc5", name="shc5")
                    nc.scalar.copy(shc[:, :], ps5[:, :])
                    for jj in range(2):
                        nc.sync.dma_start(
                            ag5_in[l][jj:jj + 1, :].rearrange("a (p c) -> (a p) c", p=128),
                            shc[:, jj * 20:(jj + 1) * 20])
                    do_allgather(ag5_in[l][:, :], ag5_out[l][:, :], 2)
                    G5mm = pool_c.tile([128, 16 * 20], F32, tag=f"g5mm{l}", name=f"g5mm{l}")
                    nc.sync.dma_start(
                        G5mm[:, :].rearrange("p (t c) -> p t c", t=NT),
                        ag5_out[l].rearrange("t (p c) -> p t c", p=128),
                    )
                    cfl = coef[l - 1]
                    G5n = pool_c.tile([128, 16 * 20], F32, tag=f"g5n{l}", name=f"g5n{l}")
                    for t in range(NT):
                        gsl = G5mm[:, t * 20:(t + 1) * 20]
                        hi_sl = gsl[:, 10:20]
                        lo_sl = gsl[:, 0:10]
                        A3 = pool_abcd.tile([128, 10], F32, tag="A5", name="A5")
                        B3 = pool_abcd.tile([128, 10], F32, tag="B5", name="B5")
                        A2 = pool_abcd.tile([128, 10], F32, tag="C5", name="C5")
                        B2 = pool_abcd.tile([128, 10], F32, tag="D5", name="D5")
                        relu_pass(A3[:, :], hi_sl, cfl["c1"][:, t:t + 1])
                        nc.vector.tensor_scalar(B3[:, :], hi_sl, 0.0, cfl["nc2"][:, t:t + 1], ALU.min, ALU.mult)
                        relu_pass(A2[:, :], lo_sl, cfl["c2"][:, t:t + 1])
                        nc.vector.tensor_scalar(B2[:, :], lo_sl, 0.0, cfl["nc1"][:, t:t + 1], ALU.min, ALU.mult)
                        nc.vector.tensor_tensor(G5n[:, t * 20:t * 20 + 10],
                                                A2[:, :], B2[:, :], ALU.subtract)
                        nc.vector.tensor_tensor(G5n[:, t * 20 + 10:t * 20 + 20],
                                                A3[:, :], B3[:, :], ALU.subtract)
                        bb5.mm(2, A3[:, :], cfl["rhv"][:, t:t + 1])
                        bb5.mm(0, B2[:, :], cfl["nrhv"][:, t:t + 1])
                    G5s = G5n
                else:
                    ps5 = pool_ps5.tile([128, 20], F32, tag="ps5f", name="ps5f")
                    wb = pool_w.tile([128, 2048], F32, tag="wb", name="wb")
                    nc.sync.dma_start(
                        wb[:, :].rearrange("p (t c) -> p t c", t=NT),
                        W1sh5[0].rearrange("(t p) c -> p t c", p=128),
                    )
                    for k in range(NT):
                        nc.tensor.matmul(
                            ps5[:, :],
                            wb[:, k * 128:(k + 1) * 128],
                            G5s[:, k * 20:(k + 1) * 20],
                            start=(k == 0), stop=(k == NT - 1))
                    shc = pool_misc.tile([128, 20], F32, tag="shc5f", name="shc5f")
                    nc.scalar.copy(shc[:, :], ps5[:, :])
                    nc.sync.dma_start(
                        ag5f_in[0:1, :].rearrange("a (p c) -> (a p) c", p=128),
                        shc[:, :])
                    do_allgather(ag5f_in[:, :], ag5f_out[:, :], 1)
                    G5f = pool_c.tile([128, NT1 * 20], F32, tag="g5f", name="g5f")
                    nc.sync.dma_start(
                        G5f[:, :].rearrange("p (t c) -> p t c", t=NT1),
                        ag5f_out[0:NT1].rearrange("t (p c) -> p t c", p=128),
                    )
                    for t in range(NT1):
                        gsl = G5f[:, t * 20:(t + 1) * 20]
                        Ph = pool_abcd.tile([128, 10], F32, tag="A5", name="A5")
                        Nh = pool_abcd.tile([128, 10], F32, tag="B5", name="B5")
                        Pl = pool_abcd.tile([128, 10], F32, tag="C5", name="C5")
                        Nl = pool_abcd.tile([128, 10], F32, tag="D5", name="D5")
                        relu_pass(Ph[:, :], gsl[:, 10:20], 1.0)
                        nc.vector.tensor_scalar(Nh[:, :], gsl[:, 10:20], 0.0, -1.0, ALU.min, ALU.mult)
                        relu_pass(Pl[:, :], gsl[:, 0:10], 1.0)
                        nc.vector.tensor_scalar(Nl[:, :], gsl[:, 0:10], 0.0, -1.0, ALU.min, ALU.mult)
                        last = t == NT1 - 1
                        bb5.mm(0, Pl[:, :], lo0n[:, t:t + 1])
                        bb5.mm(0, Nl[:, :], nhi0n[:, t:t + 1], stop=last)
                        bb5.mm(2, Ph[:, :], hi0n[:, t:t + 1])
                        bb5.mm(2, Nh[:, :], nlo0n[:, t:t + 1], stop=last)

            # final outputs: out[0]=x5, out[1]=low5, out[2]=high5
            fin = pool_misc.tile([10, 3], F32, tag="fin", name="fin")
            nc.vector.tensor_tensor(fin[:, 0:1], bb5.t[0:10, 4:5], b5t[:, :], ALU.add)
            nc.vector.tensor_tensor(fin[:, 1:2], bb5.t[0:10, 0:1], b5t[:, :], ALU.add)
            nc.vector.tensor_tensor(fin[:, 2:3], bb5.t[0:10, 2:3], b5t[:, :], ALU.add)
            nc.sync.dma_start(out_d.rearrange("k p -> p k"), fin[:, :])

    _split_multiwaits(nc)
    return nc


def make_in_maps(x, low, high, Ws, bs):
    """Host-side shard/layout prep. Ws/bs: dicts 1..5."""
    def pad_vec(v):
        p = np.zeros(IN1P, np.float32)
        p[:784] = v.reshape(-1)
        return np.ascontiguousarray(p.reshape(NT1, 128))

    xlh = np.stack([pad_vec(x), pad_vec(low), pad_vec(high)])  # [3,7,128]

    W1p = np.zeros((2048, IN1P), np.float32)
    W1p[:, :784] = Ws[1]
    common = {"xlh": xlh, "W5T": np.ascontiguousarray(Ws[5].T),
              "b5": np.ascontiguousarray(bs[5].reshape(10, 1))}
    for l, W in ((1, W1p), (2, Ws[2]), (3, Ws[3])):
        nj = W.shape[1] // 128
        common[f"W{l}t"] = np.ascontiguousarray(
            W.reshape(2048, nj, 128).transpose(1, 0, 2))
    W4t = np.ascontiguousarray(Ws[4].reshape(2048, 16, 128).transpose(1, 0, 2))
    W2t, W3t = common["W2t"], common["W3t"]
    for l in (1, 2, 3, 4):
        common[f"bc{l}"] = np.ascontiguousarray(bs[l].reshape(16, 128))

    maps = []
    for d in range(N_CORES):
        m = dict(common)
        sh = slice(256 * d, 256 * (d + 1))
        for i, W in ((2, Ws[2]), (3, Ws[3]), (4, Ws[4])):
            m[f"G{i}T"] = np.ascontiguousarray(W[sh, :].T)
        m["G1T"] = np.ascontiguousarray(W1p[sh, :].T)
        for i in (1, 2, 3, 4):
            m[f"b{i}sh"] = np.ascontiguousarray(bs[i][sh].reshape(2, 128).T)
        m["W2sh"] = np.ascontiguousarray(W2t[2 * d:2 * d + 2])
        m["W3sh"] = np.ascontiguousarray(W3t[2 * d:2 * d + 2])
        m["W4sh"] = np.ascontiguousarray(W4t[2 * d:2 * d + 2])
        m["W1sh5"] = np.ascontiguousarray(common["W1t"][min(d, 6):min(d, 6) + 1])
        maps.append(m)
    return maps


def _prep_inputs(inputs):
    Ws = {i: np.asarray(inputs[f"W{i}"], np.float32) for i in range(1, 6)}
    bs = {i: np.asarray(inputs[f"b{i}"], np.float32) for i in range(1, 6)}
    return make_in_maps(
        np.asarray(inputs["x"], np.float32),
        np.asarray(inputs["low"], np.float32),
        np.asarray(inputs["high"], np.float32),
        Ws, bs,
    )


def kernel(**inputs):
    from concourse import bass_utils

    if "nc" not in _CACHE:
        _CACHE["nc"] = build_nc()
    nc = _CACHE["nc"]

    in_maps = _prep_inputs(inputs)
    res = bass_utils.run_bass_kernel_spmd(nc, in_maps,
                                          core_ids=list(range(N_CORES)))
    out = res.results[0]["out"]
    return out[0].copy(), out[1].copy(), out[2].copy()


if __name__ == "__main__":
    import reference

    inp = reference.setup_inputs()
    inp_np = {k: np.asarray(v) for k, v in inp.items()}
    got = kernel(**inp_np)
    exp = reference.reference(**inp)
    for name, g, e in zip(("x", "low", "high"), got, exp):
        e = np.asarray(e)
        err = np.abs(g - e).max() / max(np.abs(e).max(), 1e-9)
        print(f"{name}: rel_err={err:.3e}")
        print("  got:", g[:5])
        print("  exp:", e[:5])

